# revision 1
# baseline (speedup 1.0000x reference)
"""AttentionBlock kernel for Trainium2 (8 NeuronCores, batch-sharded).

Per sample b:
    q = Wq @ x + bq            [32, N]
    k = Wk @ x + bk            [32, N]
    v = Wv @ x + bv            [256, N]
    attn = softmax(q^T k)      [N, N] (softmax over keys)
    out = gamma * (v @ attn^T) + x

Key structure:
  - S^T [keys, queries] is produced directly (row-packed 4x matmuls);
    the softmax denominator comes from ones-matmuls col-packed 4x;
    normalization is deferred to the [256, N] output.  No max
    subtraction: logits stay within +-30, inside fp32/bf16 exp range.
  - q lands replicated in all four 32-partition bands for free by
    host-tiling Wq 4x; k lands in its quad layout via host-tiled Wk
    plus four per-band PSUM->SBUF bias-copies (no SBUF scatter DMAs).
  - gamma*bv folds into the residual input host-side (exact through
    softmax normalization up to O(gamma^2) terms).
  - exp splits across ScalarE (LUT) and DVE (Schraudolph bit trick:
    i16 = rint(A*s+B) reinterpreted as bf16, ~3% relative error that
    softmax normalization mostly cancels).
  - x and weights ship as bf16 (half the HBM traffic); DMA triggers
    cost ~0.7us each on an engine queue, so they are batched ~128KB
    and split across the Sync and GpSimd queues.  Dummy matmuls at t=0
    lift the PE HAM clock gate before real work arrives.
"""

from contextlib import ExitStack

import numpy as np

import concourse.bass as bass
import concourse.mybir as mybir
import concourse.tile as tile
from concourse import bacc
from concourse.bass_utils import run_bass_kernel_spmd

B, C, H, W = 8, 256, 64, 64
N = H * W        # 4096
D = 32           # C // 8
NCORES = 8
P = 128
F32 = mybir.dt.float32
F32R = mybir.dt.float32r
BF16 = mybir.dt.bfloat16
I16 = mybir.dt.int16

NW = 8           # n-chunks of 512 queries
NCH = N // NW    # 512
MP = N // P      # 32 key-chunks of 128
QUAD = 4         # key-chunks per group (row/col packed)
NG = MP // QUAD  # 8 groups

_LN2 = float(np.log(2.0))
EXPA = 128.0 / _LN2
EXPB = 127.0 * 128.0 - 5.568

# per-(group, half) exp engine: 'a' = ScalarE, 'd' = DVE bit-trick.
EXP_SCHED = ["aa", "ad", "da", "ad", "da", "ad", "da", "ad"]


def build_bass():
    nc = bacc.Bacc("TRN2", target_bir_lowering=False, debug=False,
                   enable_asserts=False, num_devices=NCORES)

    x_d = nc.dram_tensor("x", [C, N], BF16, kind="ExternalInput").ap()
    wqT4_d = nc.dram_tensor("wqT4", [C, P], BF16, kind="ExternalInput").ap()
    wkT4_d = nc.dram_tensor("wkT4", [C, P], BF16, kind="ExternalInput").ap()
    wvT_d = nc.dram_tensor("wvT", [C, C], BF16, kind="ExternalInput").ap()
    bq4_d = nc.dram_tensor("bq4", [P, 1], F32, kind="ExternalInput").ap()
    bk4_d = nc.dram_tensor("bk4", [P, 1], F32, kind="ExternalInput").ap()
    igam_d = nc.dram_tensor("igam", [P, 1], F32, kind="ExternalInput").ap()
    ones16_d = nc.dram_tensor("ones16", [P, D], BF16, kind="ExternalInput").ap()
    ones32_d = nc.dram_tensor("ones32", [P, P], BF16, kind="ExternalInput").ap()
    out_d = nc.dram_tensor("out", [C, N], F32, kind="ExternalOutput").ap()

    with tile.TileContext(nc) as tc, ExitStack() as ctx:
        const = ctx.enter_context(tc.tile_pool(name="const", bufs=1))
        xp = ctx.enter_context(tc.tile_pool(name="xp", bufs=1))
        qk = ctx.enter_context(tc.tile_pool(name="qk", bufs=1))
        vt = ctx.enter_context(tc.tile_pool(name="vt", bufs=1))
        pt = ctx.enter_context(tc.tile_pool(name="pt", bufs=7))
        op = ctx.enter_context(tc.tile_pool(name="op", bufs=2))
        ps_st = ctx.enter_context(tc.tile_pool(name="ps_st", bufs=2, space="PSUM"))
        ps_out = ctx.enter_context(tc.tile_pool(name="ps_out", bufs=2, space="PSUM"))
        ps_den = ctx.enter_context(tc.tile_pool(name="ps_den", bufs=2, space="PSUM"))

        # two DMA trigger queues (each dma_start costs ~0.7us of queue
        # time): sync and the otherwise-idle gpsimd
        trig = [nc.sync, nc.gpsimd]
        tctr = [0]

        def dma(out, in_):
            trig[tctr[0] & 1].dma_start(out=out, in_=in_)
            tctr[0] += 1

        # critical small weights ride the otherwise-idle Scalar queue
        # (6 triggers, done before ScalarE's first bias-add) so the x
        # pieces lead the Sync/GpSimd trigger queues; ones32 is not
        # needed until the first den_b (~35us) so it loads last
        wqT4_sb = const.tile([P, 2, P], BF16)
        for ci in range(2):
            nc.scalar.dma_start(out=wqT4_sb[:, ci, :],
                                in_=wqT4_d[ci * P:(ci + 1) * P, :])
        bq4_sb = const.tile([P, 1], F32)
        nc.scalar.dma_start(out=bq4_sb, in_=bq4_d)
        wkT4_sb = const.tile([P, 2, P], BF16)
        for ci in range(2):
            nc.scalar.dma_start(out=wkT4_sb[:, ci, :],
                                in_=wkT4_d[ci * P:(ci + 1) * P, :])
        bk4_sb = const.tile([P, 1], F32)
        nc.scalar.dma_start(out=bk4_sb, in_=bk4_d)
        x_sb = xp.tile([P, 2, N], BF16)           # [128, c-half, 4096]

        def load_x_chunk(j, split):
            sl = slice(j * NCH, (j + 1) * NCH)
            for ci in range(2):
                for h in range(split):
                    hp = P // split
                    rs = slice(hp * h, hp * (h + 1))
                    dma(x_sb[rs, ci, sl],
                        x_d[ci * P + hp * h:ci * P + hp * (h + 1), sl])

        load_x_chunk(0, 4)
        load_x_chunk(1, 2)
        load_x_chunk(2, 2)
        wvT_sb = const.tile([P, 2, C], BF16)
        for ci in range(2):
            dma(wvT_sb[:, ci, :], wvT_d[ci * P:(ci + 1) * P, :])
        igam_sb = const.tile([P, 1], F32)
        dma(igam_sb, igam_d)
        ones16_sb = const.tile([P, D], BF16)
        dma(ones16_sb, ones16_d)
        ones32_sb = const.tile([P, P], BF16)      # value 1/32
        dma(ones32_sb, ones32_d)
        for j in range(3, NW):
            load_x_chunk(j, 2)

        # ---- PE warm-up: junk matmuls (on a memset tile, so no DMA
        # dependency) lift the HAM clock gate while the x DMA streams ----
        warm_in = const.tile([P, P], BF16)
        nc.vector.memset(warm_in, 0.5)
        warm_ps = ps_den.tile([P, P], F32, name="warm", tag="den")
        for w in range(12):
            nc.tensor.matmul(warm_ps, lhsT=warm_in, rhs=warm_in,
                             start=(w == 0), stop=(w == 11))
        warm_sink = const.tile([1, 1], F32)
        nc.vector.tensor_copy(out=warm_sink, in_=warm_ps[0:1, 0:1])

        q_pack = qk.tile([P, N], BF16)
        k_pack = qk.tile([P, NG, P], BF16)
        vT16_sb = vt.tile([P, MP, C], BF16)       # [128, m-chunk, 256]

        warm_in2 = const.tile([P, NCH], BF16)
        nc.vector.memset(warm_in2, 0.5)

        def proj_q(j):
            """q projection for chunk j (issued as its x lands)."""
            sl = slice(j * NCH, (j + 1) * NCH)
            ps_q = ps_out.tile([P, NCH], F32, name=f"ps_q_{j}", tag="outq")
            if 1 <= j <= 2:
                # keep the HAM clock gate warm across x-DMA waits; the
                # real q matmul below resets the bank via start=True
                for w in range(2):
                    nc.tensor.matmul(ps_q, lhsT=warm_in, rhs=warm_in2,
                                     start=True, stop=True)
            for ci in range(2):
                nc.tensor.matmul(ps_q, lhsT=wqT4_sb[:, ci, :],
                                 rhs=x_sb[:, ci, sl],
                                 start=(ci == 0), stop=(ci == 1))
            nc.scalar.add(q_pack[:, sl], ps_q, bq4_sb)

        def proj_k(j):
            # k: replicated bands from wkT4; band mi keeps only its
            # 128-key slice -> quad layout with no scatter DMA
            sl = slice(j * NCH, (j + 1) * NCH)
            ps_k = ps_out.tile([P, NCH], F32, name=f"ps_k_{j}", tag="outq")
            for ci in range(2):
                nc.tensor.matmul(ps_k, lhsT=wkT4_sb[:, ci, :],
                                 rhs=x_sb[:, ci, sl],
                                 start=(ci == 0), stop=(ci == 1))
            for mi in range(QUAD):
                pb = slice(D * mi, D * (mi + 1))
                fs = slice(P * mi, P * (mi + 1))
                if mi < 1:
                    nc.scalar.add(k_pack[pb, j, :], ps_k[pb, fs], bk4_sb[pb])
                else:
                    nc.vector.tensor_scalar_add(out=k_pack[pb, j, :],
                                                in0=ps_k[pb, fs],
                                                scalar1=bk4_sb[pb])

        def proj_v(j):
            # v projection for this chunk's 4 m-chunks (bias folded out)
            for vh in range(2):
                ps_v = ps_den.tile([P, 2, C], F32, name=f"ps_v_{j}_{vh}",
                                   tag="den")
                for mi in range(2):
                    m = 4 * j + 2 * vh + mi
                    msl = slice(m * P, (m + 1) * P)
                    for ci in range(2):
                        nc.tensor.matmul(ps_v[:, mi, :],
                                         lhsT=x_sb[:, ci, msl],
                                         rhs=wvT_sb[:, ci, :],
                                         start=(ci == 0), stop=(ci == 1))
                dstv = vT16_sb[:, 4 * j + 2 * vh:4 * j + 2 * vh + 2, :]
                if vh == 0:
                    nc.scalar.copy(dstv, ps_v)
                else:
                    nc.vector.tensor_copy(out=dstv, in_=ps_v)

        def proj(j):
            proj_q(j)
            proj_k(j)
            proj_v(j)

        # ---- attention pipeline building blocks ----
        NT = NW * NG
        pend = {}
        state = {}

        def nsl_of(n):
            return slice(n * NCH, (n + 1) * NCH)

        def st_exp(t):
            """S^T quad + exp dispatch for flat group t."""
            n_s, g_s = divmod(t, NG)
            st_a = ps_st.tile([P, 2, NCH], F32, tag="stq")
            st_b = ps_st.tile([P, 2, NCH], F32, tag="stq")
            for j in range(QUAD):
                dst = st_a if j < 2 else st_b
                nc.tensor.matmul(dst[:, j % 2, :],
                                 lhsT=k_pack[D * j:D * (j + 1), g_s, :],
                                 rhs=q_pack[D * j:D * (j + 1), nsl_of(n_s)],
                                 start=True, stop=True,
                                 tile_position=(D * j, 0))
            p_a = pt.tile([P, 2, NCH], BF16)
            p_b = pt.tile([P, 2, NCH], BF16)
            sched = "ad" if t == 0 else EXP_SCHED[g_s]
            for st, p_t, eng in ((st_a, p_a, sched[0]),
                                 (st_b, p_b, sched[1])):
                if eng == "a":
                    nc.scalar.activation(
                        out=p_t, in_=st,
                        func=mybir.ActivationFunctionType.Exp)
                else:
                    nc.vector.tensor_scalar(
                        out=p_t.bitcast(I16), in0=st,
                        scalar1=EXPA, scalar2=EXPB,
                        op0=mybir.AluOpType.mult,
                        op1=mybir.AluOpType.add)
            pend[t] = (p_a, p_b)

        def den_close(tp):
            """Last-group den quad + scale for chunk tp//NG (issued
            before the final PV group so den_b/recip overlap it)."""
            n_p = tp // NG
            den_ps = state[("den", n_p)]
            p_a, p_b = pend[tp]
            for j in range(QUAD):
                prhs = (p_a if j < 2 else p_b)[:, j % 2, :]
                nc.tensor.matmul(den_ps[D * j:D * (j + 1), :],
                                 lhsT=ones16_sb, rhs=prhs,
                                 start=False, stop=True,
                                 tile_position=(0, D * j))
            den_sb = op.tile([P, NCH], BF16, tag="dsb",
                             name=f"den_sb_{n_p}")
            nc.scalar.mul(den_sb, den_ps, igam_sb)
            state[("dsb", n_p)] = den_sb

        def pv(tp):
            n_p, g_p = divmod(tp, NG)
            first = (g_p == 0)
            last = (g_p == NG - 1)
            p_a, p_b = pend.pop(tp)
            if first:
                state[("out", n_p)] = [
                    ps_out.tile([P, NCH], F32, tag="outq",
                                name=f"out_{n_p}_{hh}") for hh in range(2)]
                state[("den", n_p)] = ps_den.tile([P, NCH], F32, tag="den",
                                                  name=f"den_{n_p}")
            out_psh = state[("out", n_p)]
            den_ps = state[("den", n_p)]
            for j in range(QUAD):
                m = g_p * QUAD + j
                prhs = (p_a if j < 2 else p_b)[:, j % 2, :]
                for hh in range(2):
                    nc.tensor.matmul(
                        out_psh[hh],
                        lhsT=vT16_sb[:, m, hh * P:(hh + 1) * P],
                        rhs=prhs,
                        start=(first and j == 0),
                        stop=(last and j == QUAD - 1))
                if last and j == 1:
                    den_b = ps_den.tile([P, NCH], F32, tag="den",
                                        name=f"den_b_{n_p}")
                    nc.tensor.matmul(den_b, lhsT=ones32_sb,
                                     rhs=state[("dsb", n_p)],
                                     start=True, stop=True)
                    rd_sb = op.tile([P, NCH], F32, name=f"rd_{n_p}")
                    nc.vector.reciprocal_approx_fast(out=rd_sb, in_=den_b)
                    state[("rd", n_p)] = rd_sb
            if not last:
                for j in range(QUAD):
                    prhs = (p_a if j < 2 else p_b)[:, j % 2, :]
                    nc.tensor.matmul(den_ps[D * j:D * (j + 1), :],
                                     lhsT=ones16_sb, rhs=prhs,
                                     start=first, stop=False,
                                     tile_position=(0, D * j))
            if last:
                # normalize on DVE (frees each PSUM bank as its mul
                # retires); residual add + store on the idle gpsimd
                rd_sb = state[("rd", n_p)]
                out_sb = op.tile([P, 2, NCH], F32, name=f"osb_{n_p}")
                lastn = (n_p == NW - 1)
                tailq = [nc.sync, nc.scalar, nc.sync, nc.scalar]
                for hh in range(2):
                    nc.vector.tensor_mul(out=out_sb[:, hh, :],
                                         in0=out_psh[hh], in1=rd_sb)
                    if lastn:
                        nc.vector.tensor_add(out=out_sb[:, hh, :],
                                             in0=out_sb[:, hh, :],
                                             in1=x_sb[:, hh, nsl_of(n_p)])
                for hh in range(2):
                    if not lastn:
                        nc.gpsimd.tensor_add(out=out_sb[:, hh, :],
                                             in0=out_sb[:, hh, :],
                                             in1=x_sb[:, hh, nsl_of(n_p)])
                    split = 4 if lastn else 1
                    hp = P // split
                    for h in range(split):
                        eng = tailq[(hh * split + h) % 4] if lastn else nc.sync
                        eng.dma_start(
                            out=out_d[hh * P + hp * h:hh * P + hp * (h + 1),
                                      nsl_of(n_p)],
                            in_=out_sb[hp * h:hp * (h + 1), hh, :])

        proj(0)
        proj(1)
        st_exp(0)
        for j in range(2, NW):
            proj(j)
        for t in range(1, NT):
            if (t - 1) % NG == NG - 1:
                den_close(t - 1)
            st_exp(t)
            pv(t - 1)
        den_close(NT - 1)
        pv(NT - 1)
    nc.compile()
    return nc


_NC_CACHE = None


def _get_nc():
    global _NC_CACHE
    if _NC_CACHE is None:
        _NC_CACHE = build_bass()
    return _NC_CACHE


def _in_maps(inputs):
    import ml_dtypes
    bf = ml_dtypes.bfloat16
    x = np.ascontiguousarray(np.asarray(inputs["x"], dtype=np.float32))
    wqT = np.ascontiguousarray(np.asarray(inputs["Wq"], np.float32).T)
    wkT = np.ascontiguousarray(np.asarray(inputs["Wk"], np.float32).T)
    wvT = np.ascontiguousarray(np.asarray(inputs["Wv"], np.float32).T)
    bq = np.asarray(inputs["bq"], np.float32)
    bk = np.asarray(inputs["bk"], np.float32)
    bv = np.asarray(inputs["bv"], np.float32)
    gamma = float(np.asarray(inputs["gamma"], np.float32).reshape(()))
    sg = 1.0 if gamma >= 0 else -1.0
    wqT4 = np.ascontiguousarray(np.tile(wqT, (1, 4)).astype(bf))   # [C, 128]
    wkT4 = np.ascontiguousarray(np.tile(wkT, (1, 4)).astype(bf))
    bq4 = np.ascontiguousarray(np.tile(bq, 4).reshape(P, 1))
    bk4 = np.ascontiguousarray(np.tile(bk, 4).reshape(P, 1))
    wvT8 = np.ascontiguousarray((wvT * sg).astype(bf))
    igam = np.full((P, 1), 1.0 / max(abs(gamma), 1e-12), np.float32)
    ones16 = np.ones((P, D), np.float32).astype(bf)
    ones32 = np.full((P, P), 1.0 / 32.0, np.float32).astype(bf)
    # fold gamma*bv into the residual input (exact through softmax
    # normalization; perturbs projections only at O(gamma^2))
    xr = (x.reshape(B, C, N) + (gamma * bv)[None, :, None]).astype(bf)
    maps = []
    for b in range(NCORES):
        maps.append({
            "x": np.ascontiguousarray(xr[b]),
            "wqT4": wqT4, "wkT4": wkT4, "wvT": wvT8,
            "bq4": bq4, "bk4": bk4, "igam": igam,
            "ones16": ones16, "ones32": ones32,
        })
    return maps


def _run(inputs, **kw):
    nc = _get_nc()
    res = run_bass_kernel_spmd(nc, _in_maps(inputs), core_ids=list(range(NCORES)),
                               **kw)
    outs = [res.results[b]["out"].reshape(C, H, W) for b in range(NCORES)]
    return np.stack(outs, axis=0).astype(np.float32), res


def kernel(**inputs) -> np.ndarray:
    out, _ = _run(inputs)
    return out

